# revision 17
# baseline (speedup 1.0000x reference)
"""Trainium2 Bass kernel for the ergodicity loss (v2: Chebyshev recurrence).

Math: for x[T=512, B=16, N=32, d=2] in [0,1]^2 and modes (k0,k1) in {0..9}^2:
    basis = cos(pi*k0*x0) * cos(pi*k1*x1)                    (separable)
    coeffs[b, k0, k1] = sum_{t,n} basis / (T*N) / nf[k1]
    loss = mean((nw * (coeffs - cd))**2)

Device strategy (8 cores, data-parallel over T: 64 timesteps/core):
  - d := sin(pi*x - pi/2) = -cos(pi*x) on ACT (arg in [-pi/2, pi/2], no
    range reduction needed). Chebyshev: T_k(d) = (-1)^k cos(k*pi*x); the
    (-1)^(k0+k1) sign is folded out on the host for free.
  - T_2 via ACT Square + affine-Copy; T_3..T_9 via the fused DVE op
    scalar_tensor_tensor: M=(T_a*2)*T_b then T=(M*1)-T_c, PAIRED two
    k-slices per instruction (bf16 4x mode, [128,1024] ops). One leaf
    pair runs on the otherwise-idle Pool engine.
  - C layout [128, 10*512] bf16, slice order pos = [T1,T0,T2,T3..T9]
    (T0 = ones via Pool memset); inner order (dd:2, f:16, b:16). This
    ordering makes every paired subtrahend a contiguous/uniform-stride AP.
  - coeffs partial sums on PE: per (f, batch-group g) matmul
    lhsT = C[dd=0] (k:10 x b:8 strided), rhs = C[dd=1] -> PSUM[80, 80]
    accumulated over 16 f-chunks; off-diagonal batch blocks ignored.
  - Output: 2x [80,80] PSUM -> DRAM DMA, host sums 8 cores + normalizes.
  - Benchmark loop: For_i over reps/U with U bodies unrolled and
    bufs=3 tile rotation so DMA latency / ACT chain / DVE chain / PE of
    successive iterations pipeline; all-engine barrier amortized 1/U.
"""
import numpy as np

T, B, NA, D = 512, 16, 32, 2
KMAX = 10
NCORES = 8
TLOC = T // NCORES          # 64 timesteps per core
KN = KMAX * KMAX
# C slice order: position -> which T_k it holds
POS2K = [0, 1, 2, 3, 4, 6, 8, 5, 7, 9]

_STATE = {}

CFG = {"unroll": 8, "bufs": 4, "cbufs": 4, "pool_pair": True}


def _np_constants():
    """Replicates reference._constants() exactly in numpy (L = ones)."""
    L = np.ones(D, dtype=np.float32)
    grids = np.meshgrid(*[np.arange(KMAX) for _ in range(D)], indexing="ij")
    K = np.stack(grids, -1).reshape(-1, D).astype(np.float32)          # [100, 2]
    k_scaled = K * np.pi / L
    nf = np.where(K[:, -1] != 0, np.sqrt(L[-1] / 2.0), 1.0).astype(np.float32)
    nw = ((1.0 + (k_scaled ** 2).sum(-1)) ** (-(D + 1) / 2.0) * 100.0).astype(np.float32)
    safe_k = np.where(K != 0, k_scaled, 1.0)
    term = np.where(K != 0,
                    (np.exp(1j * k_scaled * L) - 1.0) / (1j * safe_k * L),
                    1.0 + 0j)
    cd = (term.prod(-1).real / nf).astype(np.float32)                  # [100]
    return nf, nw, cd


def _build(reps: int = 1, loop: bool = False, cfg: dict | None = None):
    import concourse.tile as tile
    from concourse import bacc, mybir

    cfg = {**CFG, **(cfg or {})}
    f32 = mybir.dt.float32
    bf16 = mybir.dt.bfloat16
    AF = mybir.ActivationFunctionType
    OP = mybir.AluOpType
    S = 512                       # elements per k-slice
    PI = float(np.pi)
    SQ2 = float(np.sqrt(2.0))

    nc = bacc.Bacc("TRN2", target_bir_lowering=False, debug=False)
    xx = nc.dram_tensor("xx", [128, 512], f32, kind="ExternalInput").ap()
    sout = nc.dram_tensor("sout", [80, 160], f32, kind="ExternalOutput").ap()

    with tile.TileContext(nc) as tc:
        with tc.tile_pool(name="cpool", bufs=1) as cpool, \
             tc.tile_pool(name="pool", bufs=cfg["bufs"]) as pool, \
             tc.tile_pool(name="ppool", bufs=2, space="PSUM") as ppool:
            bias_sin = cpool.tile([128, 1], f32)
            nc.vector.memset(bias_sin[:], -PI / 2.0)

            # C buffers are allocated once and reused round-robin by the
            # unrolled loop bodies (cfg: unroll % cbufs == 0 so each body
            # instruction always addresses the same buffer). pos0 = T0 =
            # ones is written once here; nothing in the loop writes pos0.
            NCB = cfg["cbufs"]
            C_list = [cpool.tile([128, 10 * S], bf16, name=f"Cbuf{j}")
                      for j in range(NCB)]
            for Ct in C_list:
                CVi = Ct[:].rearrange("p (c k b) -> p c k b",
                                      c=64, k=10, b=8)
                nc.vector.memset(CVi[:, :, 0, :], 1.0)

            # C layout: (c, k, b) with c = (dd:2, f:16, g:2) = 64 blocks of
            # 80 contiguous columns (pos:10, b:8) -- matmul operands are
            # single-free-dim slices. k-slice ops use [p, c:64, inner] APs
            # (2 free dims as the compiler requires); consecutive positions
            # merge into one packed inner dim (keeps DVE 4x mode).
            # Position order POS2K = [0,1,2,3,4,6,8,5,7,9] makes every
            # grouped operand a run of consecutive positions.
            def views(Ct, Mt, At, Xt):
                CV = Ct[:].rearrange("p (c k b) -> p c k b", c=64, k=10, b=8)
                MV = Mt[:].rearrange("p (c m) -> p c m", c=64)
                AV = At[:].rearrange("p (c b) -> p c b", c=64, b=8)
                XV = Xt[:].rearrange("p (c b) -> p c b", c=64, b=8)
                return CV, MV, AV, XV

            def body(u=0):
                XX = pool.tile([128, 512], f32, tag="XX")
                C = C_list[u % NCB]
                A = pool.tile([128, S], bf16, tag="A")      # ACT square out
                M = pool.tile([128, 2 * S], bf16, tag="M")  # DVE mult scratch
                CV, MV, AV, XV = views(C, M, A, XX)

                def cs(p0):                 # single k-slice [p, 64, 8]
                    return CV[:, :, p0, :]

                def cr(p0, n):              # n consecutive positions, merged
                    v = CV[:, :, p0:p0 + n, :]
                    return v.rearrange("p c k b -> p c (k b)")

                nc.sync.dma_start(XX[:], xx)
                ps = [ppool.tile([80, 80], f32, name=f"ps{g}", tag=f"ps{g}")
                      for g in range(2)]

                # ACT chain: T1, S2 = 2*T1^2, T2 = S2 - 1
                nc.scalar.activation(cs(1), XV, AF.Sin,
                                     bias=bias_sin[:], scale=PI)
                nc.scalar.activation(AV, cs(1), AF.Square,
                                     bias=0.0, scale=SQ2)
                nc.scalar.activation(cs(2), AV, AF.Copy, bias=-1.0)

                # DVE Chebyshev (bf16 4x mode). pos: 2=T2 3=T3 4=T4 5=T6
                # 6=T8 7=T5 8=T7 9=T9
                # M3 = (T2*2)*T1 ; T3 = M3 - T1
                nc.vector.scalar_tensor_tensor(
                    MV[:, :, 0:8], cs(2), 2.0, cs(1), OP.mult, OP.mult)
                nc.vector.scalar_tensor_tensor(
                    cs(3), MV[:, :, 0:8], 1.0, cs(1), OP.mult, OP.subtract)
                # M4 = (T2*2)*T2 ; T4 = M4 - 1
                nc.vector.scalar_tensor_tensor(
                    MV[:, :, 0:8], cs(2), 2.0, cs(2), OP.mult, OP.mult)
                nc.vector.tensor_scalar(
                    cs(4), MV[:, :, 0:8], -1.0, None, OP.add)
                # [M5,M7] = ([T2,T3]*2)*[T3,T4] ; T5 = M5-T1 ; T7 = M7-T1
                nc.vector.scalar_tensor_tensor(
                    MV, cr(2, 2), 2.0, cr(3, 2), OP.mult, OP.mult)
                nc.vector.scalar_tensor_tensor(
                    cs(7), MV[:, :, 0:8], 1.0, cs(1), OP.mult, OP.subtract)
                nc.vector.scalar_tensor_tensor(
                    cs(8), MV[:, :, 8:16], 1.0, cs(1), OP.mult, OP.subtract)
                # [SQ6,SQ8] = ([T3,T4]*2)*[T3,T4] ; [T6,T8] = SQ - 1
                nc.vector.scalar_tensor_tensor(
                    MV, cr(3, 2), 2.0, cr(3, 2), OP.mult, OP.mult)
                nc.vector.tensor_scalar(
                    cr(5, 2), MV, -1.0, None, OP.add)
                # M9 = (T4*2)*T5 on DVE; leaf sub T9 = M9 - T1 on Pool
                # (Pool supports TensorTensor but not ScalarTensorTensor)
                nc.vector.scalar_tensor_tensor(
                    AV, cs(4), 2.0, cs(7), OP.mult, OP.mult)
                if cfg["pool_pair"]:
                    nc.gpsimd.tensor_tensor(cs(9), AV, cs(1), OP.subtract)
                else:
                    nc.vector.scalar_tensor_tensor(
                        cs(9), AV, 1.0, cs(1), OP.mult, OP.subtract)

                # matmuls: block (dd, f, g) = contiguous 80 cols
                for f in range(16):
                    for g in range(2):
                        nc.tensor.matmul(
                            ps[g][:],
                            C[:, (f * 2 + g) * 80:(f * 2 + g) * 80 + 80],
                            C[:, (32 + f * 2 + g) * 80:
                              (32 + f * 2 + g) * 80 + 80],
                            start=(f == 0), stop=(f == 15))

                SO = pool.tile([80, 160], f32, tag="SO")
                nc.scalar.copy(SO[:, 0:80], ps[0][:])
                nc.vector.tensor_scalar(SO[:, 80:160], ps[1][:], 1.0, None,
                                        OP.mult)
                nc.sync.dma_start(sout, SO[:])

            if loop:
                U = cfg["unroll"]
                assert reps % U == 0 and U % NCB == 0, (reps, U, NCB)
                with tc.For_i(0, reps // U, 1):
                    for u in range(U):
                        body(u)
            else:
                for u in range(reps):
                    body(u)

    nc.compile()
    return nc


def _get_state():
    if "nc" not in _STATE:
        _STATE["nc"] = _build()
    return _STATE["nc"]


def _shard_inputs(x: np.ndarray):
    """x [512, 16, 32, 2] -> per-core {xx [128, 512]}.

    xx free layout: dd*256 + f*16 + b, partition p = tp*32 + agent where the
    64 local timesteps split as (f:16, tp:4).
    """
    in_maps = []
    for c in range(NCORES):
        xc = x[c * TLOC:(c + 1) * TLOC]            # [64, 16, 32, 2]
        arr = xc.reshape(16, 4, 16, 32, 2)         # (f, tp, b, a, d)
        arr = arr.transpose(4, 1, 3, 0, 2)         # (d, tp, a, f, b)
        arr = arr.reshape(2, 128, 256)             # p = tp*32+a, free = f*16+b
        xxc = np.concatenate([arr[0], arr[1]], axis=1)
        in_maps.append({"xx": np.ascontiguousarray(xxc)})
    return in_maps


def _gather(souts):
    """souts: list of 8 [80, 160] partials -> scalar loss (float32).

    sout row m = pos0*8 + b', col (80*g + pos1*8 + b'') for batch b = 8*g+b'.
    Values hold (-1)^(k0+k1) cos*cos sums with k = POS2K[pos].
    """
    total = np.zeros((80, 160), dtype=np.float64)
    for s in souts:
        total += s.astype(np.float64)
    Sm = np.empty((B, KMAX, KMAX), dtype=np.float64)
    perm = np.array(POS2K)
    for g in range(2):
        for bp in range(8):
            blk = total[bp::8, 80 * g + bp:80 * (g + 1):8]   # [pos0, pos1]
            # un-permute positions -> k order
            inv = np.empty(KMAX, dtype=np.int64)
            inv[perm] = np.arange(KMAX)
            Sm[8 * g + bp] = blk[inv][:, inv]
    # sign fix: slice pos holds (-1)^k cos(k pi x)
    sg = (-1.0) ** (np.arange(KMAX)[:, None] + np.arange(KMAX)[None, :])
    Sm = Sm * sg
    nf, nw, cd = _np_constants()
    coeffs = Sm.reshape(B, KN) / (NA * T) / nf[None, :].astype(np.float64)
    d = nw[None, :].astype(np.float64) * (coeffs - cd[None, :].astype(np.float64))
    loss = np.mean(d * d)
    return np.float32(loss)


def kernel(x: np.ndarray) -> np.ndarray:
    from concourse.bass_utils import run_bass_kernel_spmd

    nc = _get_state()
    in_maps = _shard_inputs(np.asarray(x, dtype=np.float32))
    res = run_bass_kernel_spmd(nc, in_maps, list(range(NCORES)))
    souts = [r["sout"] for r in res.results]
    return _gather(souts)


# revision 18
# speedup vs baseline: 4.5996x; 4.5996x over previous
"""Trainium2 Bass kernel for the ergodicity loss (v3: power-moment basis).

Math: for x[T=512, B=16, N=32, d=2] in [0,1]^2 and modes (k0,k1) in {0..9}^2:
    basis = cos(pi*k0*x0) * cos(pi*k1*x1)                    (separable)
    coeffs[b, k0, k1] = sum_{t,n} basis / (T*N) / nf[k1]
    loss = mean((nw * (coeffs - cd))**2)

Device strategy (8 cores, data-parallel over T: 64 timesteps/core):
  - Instead of cos values, the device accumulates the MOMENT matrix
    Mo[b, m0, m1] = sum_{t,n} d0^m0 * d1^m1 with d = sin(pi*x - pi/2)
    = -cos(pi*x). The 10x10 Chebyshev transform cos(k*pi*x) = T_k(-d) =
    sum_m a[k,m] (-1)^m d^m is applied on the HOST (tiny [10,10] GEMMs),
    so the device only needs powers d^m -- pure multiplies, no subtracts:
      ACT:  d = Sin(pi*x - pi/2)   (arg in [-pi/2,pi/2], no range redux)
            d2 = Square(d)
      DVE:  d3 = d*d2; [d4,d6] = [d2,d3]^2; [d5,d7] = [d2,d3]*[d3,d4]
            d8 = d4^2          (tensor_tensor, bf16 2x mode)
      Pool: d9 = d4*d5         (otherwise-idle engine)
    bf16 end-to-end loss rel err ~6e-6 (verified vs numpy).
  - C layout [128, 64*80] bf16: 64 blocks (dd:2, f:16, g:2) of 80
    contiguous cols (pos:10, b:8) -- matmul operands are single-free-dim
    slices (HW requirement); elementwise ops use [p, 64, n*8] strided
    views (packed last dim). Position order pos->m = [0,1,2,3,4,6,5,7,8,9]
    makes every grouped operand/output a run of consecutive positions.
    pos0 = ones is memset ONCE per C buffer outside the loop (nothing in
    the loop writes it).
  - PE: per (f, batch-group g): lhsT = C[dd=0] block, rhs = C[dd=1] block
    -> PSUM[80, 80] accumulated over 16 f-chunks; off-diagonal batch
    blocks are garbage, ignored at gather.
  - Benchmark loop: For_i over reps/U with U=8 bodies unrolled, tile
    bufs=4 (C round-robin over 4 persistent buffers) so DMA latency,
    ACT, DVE, Pool and PE of successive iterations pipeline; the For_i
    all-engine barrier is amortized 1/U.
Host: sum 8 per-core [80,160] partials, extract diagonal batch blocks,
apply the A-transform + tiny [16,100] normalization + weighted MSE.
"""
import numpy as np

T, B, NA, D = 512, 16, 32, 2
KMAX = 10
NCORES = 8
TLOC = T // NCORES          # 64 timesteps per core
KN = KMAX * KMAX
# C slice order: position -> which power d^m it holds
POS2M = [0, 1, 2, 3, 4, 6, 5, 7, 8, 9]

_STATE = {}

CFG = {"unroll": 8, "bufs": 4, "cbufs": 4, "pool_ops": 1, "act_c2": True}


def _np_constants():
    """Replicates reference._constants() exactly in numpy (L = ones)."""
    L = np.ones(D, dtype=np.float32)
    grids = np.meshgrid(*[np.arange(KMAX) for _ in range(D)], indexing="ij")
    K = np.stack(grids, -1).reshape(-1, D).astype(np.float32)          # [100, 2]
    k_scaled = K * np.pi / L
    nf = np.where(K[:, -1] != 0, np.sqrt(L[-1] / 2.0), 1.0).astype(np.float32)
    nw = ((1.0 + (k_scaled ** 2).sum(-1)) ** (-(D + 1) / 2.0) * 100.0).astype(np.float32)
    safe_k = np.where(K != 0, k_scaled, 1.0)
    term = np.where(K != 0,
                    (np.exp(1j * k_scaled * L) - 1.0) / (1j * safe_k * L),
                    1.0 + 0j)
    cd = (term.prod(-1).real / nf).astype(np.float32)                  # [100]
    return nf, nw, cd


def _cheb_transform():
    """A'[k, m] with cos(k*pi*x) = sum_m A'[k,m] d^m, d = -cos(pi*x)."""
    A = np.zeros((KMAX, KMAX))
    A[0, 0] = 1.0
    A[1, 1] = 1.0
    for k in range(2, KMAX):
        A[k, 1:] += 2 * A[k - 1, :-1]
        A[k, :] -= A[k - 2, :]
    return A * ((-1.0) ** np.arange(KMAX))[None, :]


def _build(reps: int = 1, loop: bool = False, cfg: dict | None = None):
    import concourse.tile as tile
    from concourse import bacc, mybir

    cfg = {**CFG, **(cfg or {})}
    f32 = mybir.dt.float32
    bf16 = mybir.dt.bfloat16
    AF = mybir.ActivationFunctionType
    OP = mybir.AluOpType
    S = 512                       # elements per position slice
    PI = float(np.pi)

    nc = bacc.Bacc("TRN2", target_bir_lowering=False, debug=False)
    xx = nc.dram_tensor("xx", [128, 512], f32, kind="ExternalInput").ap()
    sout = nc.dram_tensor("sout", [80, 160], f32, kind="ExternalOutput").ap()

    with tile.TileContext(nc) as tc:
        with tc.tile_pool(name="cpool", bufs=1) as cpool, \
             tc.tile_pool(name="pool", bufs=cfg["bufs"]) as pool, \
             tc.tile_pool(name="ppool", bufs=2, space="PSUM") as ppool:
            bias_sin = cpool.tile([128, 1], f32)
            nc.vector.memset(bias_sin[:], -PI / 2.0)

            # C buffers allocated once, used round-robin by the unrolled
            # bodies (unroll % cbufs == 0 keeps each body's buffer fixed).
            # pos0 = d^0 = ones, written once; the loop never writes pos0.
            NCB = cfg["cbufs"]
            C_list = [cpool.tile([128, 10 * S], bf16, name=f"Cbuf{j}")
                      for j in range(NCB)]
            for Ct in C_list:
                CVi = Ct[:].rearrange("p (c k b) -> p c k b",
                                      c=64, k=10, b=8)
                nc.vector.memset(CVi[:, :, 0, :], 1.0)

            def body(u=0):
                XX = pool.tile([128, 512], f32, tag="XX")
                C = C_list[u % NCB]
                CV = C[:].rearrange("p (c k b) -> p c k b", c=64, k=10, b=8)
                XV = XX[:].rearrange("p (c b) -> p c b", c=64, b=8)

                def cs(p0):                 # single slice [p, 64, 8]
                    return CV[:, :, p0, :]

                def cr(p0, n):              # n consecutive positions, merged
                    v = CV[:, :, p0:p0 + n, :]
                    return v.rearrange("p c k b -> p c (k b)")

                nc.sync.dma_start(XX[:], xx)
                ps = [ppool.tile([80, 80], f32, name=f"ps{g}", tag=f"ps{g}")
                      for g in range(2)]

                # ACT: d = -cos(pi x) -> pos1 ; d^2 -> pos2
                nc.scalar.activation(cs(1), XV, AF.Sin,
                                     bias=bias_sin[:], scale=PI)
                if cfg["act_c2"]:
                    nc.scalar.activation(cs(2), cs(1), AF.Square,
                                         bias=0.0, scale=1.0)
                else:
                    nc.vector.tensor_tensor(cs(2), cs(1), cs(1), OP.mult)

                # DVE powers (bf16, 2x mode). pos: 3=d3 4=d4 5=d6 6=d5
                # 7=d7 8=d8 9=d9
                nc.vector.tensor_tensor(cs(3), cs(1), cs(2), OP.mult)
                # [d4,d6] = [d2,d3]*[d2,d3]
                nc.vector.tensor_tensor(cr(4, 2), cr(2, 2), cr(2, 2), OP.mult)
                # [d5,d7] = [d2,d3]*[d3,d4]
                nc.vector.tensor_tensor(cr(6, 2), cr(2, 2), cr(3, 2), OP.mult)
                # d8 = d4*d4 ; d9 = d4*d5 (leaf ops; d9 on the idle Pool)
                nc.vector.tensor_tensor(cs(8), cs(4), cs(4), OP.mult)
                if cfg["pool_ops"] >= 1:
                    nc.gpsimd.tensor_tensor(cs(9), cs(4), cs(6), OP.mult)
                else:
                    nc.vector.tensor_tensor(cs(9), cs(4), cs(6), OP.mult)

                # matmuls: block (dd, f, g) = contiguous 80 cols
                for f in range(16):
                    for g in range(2):
                        nc.tensor.matmul(
                            ps[g][:],
                            C[:, (f * 2 + g) * 80:(f * 2 + g) * 80 + 80],
                            C[:, (32 + f * 2 + g) * 80:
                              (32 + f * 2 + g) * 80 + 80],
                            start=(f == 0), stop=(f == 15))

                SO = pool.tile([80, 160], f32, tag="SO")
                nc.scalar.copy(SO[:, 0:80], ps[0][:])
                nc.vector.tensor_scalar(SO[:, 80:160], ps[1][:], 1.0, None,
                                        OP.mult)
                nc.sync.dma_start(sout, SO[:])

            if loop:
                U = cfg["unroll"]
                assert reps % U == 0 and U % NCB == 0, (reps, U, NCB)
                with tc.For_i(0, reps // U, 1):
                    for u in range(U):
                        body(u)
            else:
                for u in range(reps):
                    body(u)

    nc.compile()
    return nc


def _get_state():
    if "nc" not in _STATE:
        _STATE["nc"] = _build()
    return _STATE["nc"]


def _shard_inputs(x: np.ndarray):
    """x [512, 16, 32, 2] -> per-core {xx [128, 512]}.

    xx free layout: dd*256 + f*16 + b, partition p = tp*32 + agent where the
    64 local timesteps split as (f:16, tp:4).
    """
    in_maps = []
    for c in range(NCORES):
        xc = x[c * TLOC:(c + 1) * TLOC]            # [64, 16, 32, 2]
        arr = xc.reshape(16, 4, 16, 32, 2)         # (f, tp, b, a, d)
        arr = arr.transpose(4, 1, 3, 0, 2)         # (d, tp, a, f, b)
        arr = arr.reshape(2, 128, 256)             # p = tp*32+a, free = f*16+b
        xxc = np.concatenate([arr[0], arr[1]], axis=1)
        in_maps.append({"xx": np.ascontiguousarray(xxc)})
    return in_maps


def _gather(souts):
    """souts: list of 8 [80, 160] moment partials -> scalar loss (float32).

    sout row = pos0*8 + b', col (80*g + pos1*8 + b'') for batch b = 8*g+b',
    holding sum d0^m0 d1^m1 with m = POS2M[pos]. Host applies the
    Chebyshev transform A' then the reference normalization.
    """
    total = np.zeros((80, 160), dtype=np.float64)
    for s in souts:
        total += s.astype(np.float64)
    perm = np.array(POS2M)
    inv = np.empty(KMAX, dtype=np.int64)
    inv[perm] = np.arange(KMAX)
    Ap = _cheb_transform()                       # [k, m]
    Sm = np.empty((B, KMAX, KMAX), dtype=np.float64)
    for g in range(2):
        for bp in range(8):
            Mo = total[bp::8, 80 * g + bp:80 * (g + 1):8]   # [pos0, pos1]
            Mo = Mo[inv][:, inv]                             # [m0, m1]
            Sm[8 * g + bp] = Ap @ Mo @ Ap.T
    nf, nw, cd = _np_constants()
    coeffs = Sm.reshape(B, KN) / (NA * T) / nf[None, :].astype(np.float64)
    d = nw[None, :].astype(np.float64) * (coeffs - cd[None, :].astype(np.float64))
    loss = np.mean(d * d)
    return np.float32(loss)


def kernel(x: np.ndarray) -> np.ndarray:
    from concourse.bass_utils import run_bass_kernel_spmd

    nc = _get_state()
    in_maps = _shard_inputs(np.asarray(x, dtype=np.float32))
    res = run_bass_kernel_spmd(nc, in_maps, list(range(NCORES)))
    souts = [r["sout"] for r in res.results]
    return _gather(souts)
